# revision 1
# baseline (speedup 1.0000x reference)
"""AFM layer kernel for 8 Trainium2 NeuronCores (v7).

Math (per batch b, F=50 fields, E=64, A=10):
  pairs p=(i<j), inter_p = x_i * x_j
  q_c[p]  = inter_p . W[:,c],  logit l_p = sum_c h_c relu(q_c[p] + b_c)
  score = softmax_p(l),  out[b] = sum_p score_p * (inter_p . proj_p)

Reformulation: out[b] = (sum_p e_p r_p) / (sum_p e_p) with e_p = exp(l_p),
r_p = inter_p . proj_p.  Both q_c and r are bilinear forms
x_i^T diag(c) x_j, so inter [B,1225,64] is never materialized.

Engine assignment (per 2-batch iteration; 128 iterations/core; slabs of 8):
  DVE : u[(half,e),(c,j)] = xt * cexp  (moving operand build)
        ers = e * r (PSUM read)
  PE  : Q[(half,i),(c,j)] = st^T @ u   (block-diag stationary, 500 cols)
        r-column matmul (50 cols)
        c-sum l = sum_c s_c relu(q_c) + mask via accumulating
        identity-stationary matmuls (+I for positive-h c-blocks, -I for
        negative, maskt8 with +I) -- sign fold and pair-mask are free
  ACT : relu (PSUM->SBUF, batched over 2 iterations), exp (per 8-slab)
  Pool: num/den j-sums (sum-reduce [100,8,50]->[100,8], SBUF only)

|h| is folded into W columns on host; columns ordered sign-descending so
+I/-I stationary switches happen once per slab. attention_b is zero in
setup_inputs; a generic bias path (extra K=1 matmul into the Q psum)
activates only when b != 0.
"""

import os
import sys
import numpy as np

for _p in ("/opt/trn_rl_repo",):
    if _p not in sys.path:
        sys.path.insert(0, _p)

B = 2048
NCORES = 8
B_LOC = B // NCORES  # 256
NF = 50
E = 64
NA = 10
NCOL = NA + 1  # 10 W-columns + projection_p
NIT = B_LOC // 2  # 128 two-batch iterations
G = 8  # iterations per slab
NSLAB = NIT // G  # 16
NCHUNK = 16  # input DMA chunks
CH_IT = NIT // NCHUNK  # 8 iterations per chunk
NRELU_DVE = 0  # trailing c-blocks whose relu runs on DVE instead of ACT
MASK_NEG = -30.0
W5 = NCOL * NF  # 550
WQ = NA * NF  # 500 (W-part of the matmul columns)

LAST_RESULTS = None  # stash for test.py (exec_time_ns etc.)


def _build(npos, nneg, has_bias=False, rep=1):
    from contextlib import ExitStack
    import concourse.bass as bass
    import concourse.tile as tile
    from concourse import bacc, mybir

    f32 = mybir.dt.float32
    bf16 = mybir.dt.bfloat16
    AF = mybir.ActivationFunctionType
    OP = mybir.AluOpType

    nc = bacc.Bacc()
    xt_ext = nc.declare_dram_parameter("xt", [128, NIT * NF], bf16, isOutput=False)
    std_ext = nc.declare_dram_parameter("std", [128, NIT * 100], bf16, isOutput=False)
    cmat_ext = nc.declare_dram_parameter("cmat", [128, NCOL], f32, isOutput=False)
    maskt8_ext = nc.declare_dram_parameter("maskt8", [100, G * NF], bf16, isOutput=False)
    eye_ext = nc.declare_dram_parameter("eye", [100, 200], bf16, isOutput=False)
    onesfin_ext = nc.declare_dram_parameter("onesfin", [100, 2], f32, isOutput=False)
    if has_bias:
        cbias_ext = nc.declare_dram_parameter("cbias", [1, WQ], bf16, isOutput=False)
    out_ext = nc.declare_dram_parameter("out", [B_LOC, 1], f32, isOutput=True)

    with tile.TileContext(nc) as tc, ExitStack() as ctx:
        cpool = ctx.enter_context(tc.tile_pool(name="const", bufs=1))
        xpool = ctx.enter_context(tc.tile_pool(name="xin", bufs=1))
        upool = ctx.enter_context(tc.tile_pool(name="u", bufs=2))
        mpool = ctx.enter_context(tc.tile_pool(name="m", bufs=2))
        epool = ctx.enter_context(tc.tile_pool(name="e", bufs=2))
        smp = ctx.enter_context(tc.tile_pool(name="small", bufs=2))
        accp = ctx.enter_context(tc.tile_pool(name="acc", bufs=1))
        psq = ctx.enter_context(tc.tile_pool(name="psq", bufs=2, space="PSUM"))
        psr = ctx.enter_context(tc.tile_pool(name="psr", bufs=2, space="PSUM"))
        psl = ctx.enter_context(tc.tile_pool(name="psl", bufs=2, space="PSUM"))

        # ---- first input chunk before anything else (unblocks compute) ----
        xt_ch = []
        std_ch = []

        def load_chunk(c):
            xc = xpool.tile([128, CH_IT * NF], bf16, name=f"xt{c}")
            nc.sync.dma_start(xc[:], xt_ext[:, c * CH_IT * NF : (c + 1) * CH_IT * NF])
            xt_ch.append(xc)
            sc = xpool.tile([128, CH_IT * 100], bf16, name=f"std{c}")
            nc.sync.dma_start(
                sc[:], std_ext[:, c * CH_IT * 100 : (c + 1) * CH_IT * 100]
            )
            std_ch.append(sc)

        load_chunk(0)

        # ---- constants ----
        cexpS = cpool.tile([128, NCOL], f32)
        nc.gpsimd.dma_start(cexpS[:], cmat_ext[:])
        maskt8 = cpool.tile([100, G * NF], bf16)
        nc.gpsimd.dma_start(maskt8[:], maskt8_ext[:])
        eye = cpool.tile([100, 200], bf16)
        nc.gpsimd.dma_start(eye[:], eye_ext[:])
        onesfin = cpool.tile([100, 2], f32)
        nc.gpsimd.dma_start(onesfin[:], onesfin_ext[:])
        if has_bias:
            cbias = cpool.tile([1, WQ], bf16)
            nc.gpsimd.dma_start(cbias[:], cbias_ext[:])
            ones1 = cpool.tile([1, 100], bf16)
            nc.gpsimd.memset(ones1[:], 1.0)
        eyep = eye[:, 0:100]
        eyen = eye[:, 100:200]

        # ---- remaining input chunks ----
        for c in range(1, NCHUNK):
            load_chunk(c)

        acc = accp.tile([100, 2 * NIT], f32)

        # state carried between slabs for the 1-slab-delayed tail
        prev = None  # (m_sl, r_ps, s_data, ell_done)

        def pair_csum(ell, m_sl, k0):
            """csum for one iteration pair (k0, k0+1) into its own
            accumulation-group region of ell (final slab: avoids the serial
            end-of-kernel csum burst)."""
            reg = ell[:, k0 * NF : (k0 + 2) * NF]
            nc.tensor.matmul(
                reg, eyep, maskt8[:, k0 * NF : (k0 + 2) * NF],
                start=True, stop=False,
            )
            for c in range(NA):
                ey = eyep if c < npos else eyen
                rhs = m_sl[:].rearrange("p (g cj) -> p g cj", cj=WQ)[
                    :, k0 : k0 + 2, c * NF : (c + 1) * NF
                ]
                nc.tensor.matmul(reg, ey, rhs, start=False, stop=(c == NA - 1))

        def csum_mms(m_sl, part):
            """Emit csum group `part` (0..3) for slab with relu'd m_sl.

            11 accumulating matmuls (mask + 10 signed identity c-sums) split
            into 4 groups of 3/3/3/2 so the PE burst interleaves with the
            next slab's Q matmuls.
            """
            ell = csum_mms.ell
            groups = [(-1, 0, 1), (2, 3, 4), (5, 6, 7), (8, 9)]
            for c in groups[part]:
                if c < 0:
                    nc.tensor.matmul(
                        ell[:], eyep, maskt8[:], start=True, stop=False
                    )
                    continue
                ey = eyep if c < npos else eyen
                # rhs: [100, (G: stride WQ), (NF: 1)] at column offset c*NF
                rhs = m_sl[:].rearrange("p (g cj) -> p g cj", cj=WQ)[
                    :, :, c * NF : (c + 1) * NF
                ]
                nc.tensor.matmul(
                    ell[:], ey, rhs, start=False, stop=(c == NA - 1)
                )

        def emit_tail_rest(r_ps, s_data, ell_done=None):
            """exp + ers + num/den for slab s_data (after its csum mms)."""
            ell = ell_done if ell_done is not None else csum_mms.ell
            e_sl = epool.tile([100, G * NF], bf16, tag="esl")
            nc.scalar.activation(e_sl[:], ell[:], AF.Exp)
            ers = epool.tile([100, G * NF], bf16, tag="ers")
            nc.vector.tensor_tensor(ers[:], e_sl[:], r_ps[:], op=OP.mult)
            base = s_data * G
            nc.vector.reduce_sum(
                acc[:, NIT + base : NIT + base + G],
                e_sl[:].rearrange("p (g j) -> p g j", j=NF),
                axis=mybir.AxisListType.X,
            )
            nc.vector.reduce_sum(
                acc[:, base : base + G],
                ers[:].rearrange("p (g j) -> p g j", j=NF),
                axis=mybir.AxisListType.X,
            )

        for rs in range(rep * NSLAB + 1):
            last = rs == rep * NSLAB
            s = rs % NSLAB
            ch = s // (NSLAB // NCHUNK)
            xt_t = xt_ch[ch]
            std_t = std_ch[ch]
            it0 = (s % (NSLAB // NCHUNK)) * G * NF  # xt col offset in chunk
            st0 = (s % (NSLAB // NCHUNK)) * G * 100

            if not last:
                m_sl = mpool.tile([100, G * WQ], bf16, tag="msl")
                r_ps = psr.tile([100, G * NF], f32, tag="rps")
                u_sl = upool.tile([128, G * W5], bf16, tag="usl")
                # u[(half,e), (it,c,j)] = xt[(half,e), (it,j)] * Cmat[(half,e), c]
                # First slab: emit the first pair's u separately so the first
                # Q matmul (and the ACT pipeline behind it) unblocks early.
                u_groups = ((0, 2), (2, G)) if rs == 0 else ((0, G),)
                for g0, g1 in u_groups:
                    for c in range(NCOL):
                        nc.vector.tensor_scalar(
                            u_sl[:]
                            .rearrange("p (g cj) -> p g cj", cj=W5)[
                                :, g0:g1, c * NF : (c + 1) * NF
                            ],
                            xt_t[:, it0 + g0 * NF : it0 + g1 * NF].rearrange(
                                "p (g j) -> p g j", j=NF
                            ),
                            cexpS[:, c : c + 1],
                            None,
                            op0=OP.mult,
                        )
            if prev is not None and rs > 0 and prev[3] is None:
                csum_mms.ell = psl.tile([100, G * NF], f32, tag="ell")
            final_slab = rs == rep * NSLAB - 1
            final_ell = None
            q_t = None
            for k in range(G):
                if not last:
                    soff = st0 + k * 100
                    uoff = k * W5
                    if k % 2 == 0:
                        q_t = psq.tile([100, 1024], f32, tag="qt")
                    qoff = (k % 2) * 512
                    st_ap = std_t[:, soff : soff + 100]
                    if has_bias:
                        nc.tensor.matmul(
                            q_t[:, qoff : qoff + WQ], ones1[:], cbias[:],
                            start=True, stop=False,
                        )
                    nc.tensor.matmul(
                        q_t[:, qoff : qoff + WQ], st_ap, u_sl[:, uoff : uoff + WQ],
                        start=not has_bias, stop=True,
                    )
                    nc.tensor.matmul(
                        r_ps[:, k * NF : (k + 1) * NF], st_ap,
                        u_sl[:, uoff + WQ : uoff + W5],
                        start=True, stop=True,
                    )
                if prev is not None and k % 2 == 1 and prev[3] is None:
                    csum_mms(prev[0], k // 2)
                if final_slab and k % 2 == 1 and k > 1:
                    # final slab: csum the PREVIOUS pair now (its relu is
                    # done); last pair + exp handled after the loop
                    if final_ell is None:
                        final_ell = psl.tile([100, G * NF], f32, tag="ell")
                    pair_csum(final_ell, m_sl, k - 3)
                if not last and k % 2 == 1:
                    # relu two iterations: q_t[:, {0,512}+0:500] -> m_sl.
                    # Leading c-blocks on ACT, trailing NRELU_DVE blocks on DVE
                    # to balance engine load.
                    split = (NA - NRELU_DVE) * NF
                    qv = q_t[:].rearrange("p (two x) -> p two x", two=2)
                    mv = m_sl[:].rearrange("p (g cj) -> p g cj", cj=WQ)[
                        :, k - 1 : k + 1, :
                    ]
                    nc.scalar.activation(
                        mv[:, :, 0:split], qv[:, :, 0:split], AF.Relu
                    )
                    if NRELU_DVE:
                        nc.vector.tensor_scalar(
                            mv[:, :, split:WQ],
                            qv[:, :, split:WQ],
                            0.0,
                            None,
                            op0=OP.max,
                        )
            if final_slab:
                pair_csum(final_ell, m_sl, G - 2)
            if prev is not None:
                emit_tail_rest(prev[1], prev[2], prev[3])
            if not last:
                prev = (m_sl, r_ps, s, final_ell)

        # final: per-batch partition sums (num, den), divide, store.
        # The [2, 256] result lives in a spare rotation of the ell tag to
        # stay within the 8-bank PSUM budget.
        pft = psl.tile([100, G * NF], f32, tag="ell")
        pf = pft[0:2, 0 : 2 * NIT]
        nc.tensor.matmul(pf, onesfin[:], acc[:], start=True, stop=True)
        rcp = smp.tile([2, NIT], f32, tag="rcp")
        nc.vector.reciprocal(rcp[:], pf[:, NIT : 2 * NIT])
        res = smp.tile([2, NIT], f32, tag="res")
        nc.vector.tensor_tensor(res[:], pf[:, 0:NIT], rcp[:], op=OP.mult)
        out_v = out_ext[:].rearrange("(i m) o -> m (i o)", m=2)
        nc.sync.dma_start(out_v, res[:])

    nc.compile()
    return nc


def _host_prep(x, attention_W, attention_b, projection_h, projection_p):
    import ml_dtypes

    bf = ml_dtypes.bfloat16
    x = np.ascontiguousarray(np.asarray(x, dtype=np.float32))
    W = np.asarray(attention_W, dtype=np.float32)
    bv = np.asarray(attention_b, dtype=np.float32)
    h = np.asarray(projection_h, dtype=np.float32).reshape(-1)
    p = np.asarray(projection_p, dtype=np.float32).reshape(-1)

    habs = np.abs(h)
    sgn = np.sign(h)
    order = np.argsort(-sgn, kind="stable")
    Wp = (W * habs[None, :])[:, order]
    bp = (bv * habs)[order]
    npos = int((sgn[order] > 0).sum())
    has_bias = bool(np.any(bv != 0.0))

    Cmat = np.concatenate([Wp, p[:, None]], axis=1)  # [64, 11]
    cmat = np.ascontiguousarray(
        np.concatenate([Cmat, Cmat], axis=0), dtype=np.float32
    )  # [128, 11]
    cbias = np.repeat(bp, NF).reshape(1, WQ).astype(bf)  # [1,500]
    im = np.arange(NF)
    m50 = np.where(im[:, None] < im[None, :], 0.0, MASK_NEG).astype(np.float32)
    maskt = np.concatenate([m50, m50], axis=0)  # [100, 50]
    maskt8 = np.ascontiguousarray(np.tile(maskt, (1, G))).astype(bf)  # [100,400]
    eye = np.zeros((100, 200), dtype=np.float32)
    eye[:, 0:100] = np.eye(100)
    eye[:, 100:200] = -np.eye(100)
    eye = eye.astype(bf)
    onesfin = np.zeros((100, 2), dtype=np.float32)
    onesfin[0:50, 0] = 1.0
    onesfin[50:100, 1] = 1.0

    # per-core packed layouts
    xcores = x.reshape(NCORES, B_LOC, NF, E)
    xt_bf = np.transpose(xcores, (0, 1, 3, 2)).astype(bf)  # [C, 256, 64, 50]
    xt_it = xt_bf.reshape(NCORES, NIT, 2, E, NF)  # [C, it, half, e, j]
    # xt [128, NIT*50]: rows (half, e), cols (it, j)
    xt_all = np.ascontiguousarray(
        xt_it.transpose(0, 2, 3, 1, 4).reshape(NCORES, 128, NIT * NF)
    )
    # std [128, NIT*100]: block-diag per iteration
    std_all = np.zeros((NCORES, 2, E, NIT, 2, NF), dtype=bf)  # [C,rh,e,it,ch,j]
    std_all[:, 0, :, :, 0, :] = xt_it[:, :, 0].transpose(0, 2, 1, 3)
    std_all[:, 1, :, :, 1, :] = xt_it[:, :, 1].transpose(0, 2, 1, 3)
    std_all = np.ascontiguousarray(std_all.reshape(NCORES, 128, NIT * 100))

    return npos, has_bias, xt_all, std_all, cmat, cbias, maskt8, eye, onesfin


def kernel(x, attention_W, attention_b, projection_h, projection_p):
    global LAST_RESULTS
    from concourse.bass_utils import run_bass_kernel_spmd

    npos, has_bias, xt_all, std_all, cmat, cbias, maskt8, eye, onesfin = _host_prep(
        x, attention_W, attention_b, projection_h, projection_p
    )
    nc = _build(npos, NA - npos, has_bias=has_bias)

    in_maps = []
    for c in range(NCORES):
        m = {
            "xt": xt_all[c],
            "std": std_all[c],
            "cmat": cmat,
            "maskt8": maskt8,
            "eye": eye,
            "onesfin": onesfin,
        }
        if has_bias:
            m["cbias"] = cbias
        in_maps.append(m)
    trace = os.environ.get("BASS_KERNEL_TRACE", "0") == "1"
    res = run_bass_kernel_spmd(nc, in_maps, core_ids=list(range(NCORES)), trace=trace)
    LAST_RESULTS = res
    outs = [np.asarray(r["out"]).reshape(B_LOC, 1) for r in res.results]
    return np.concatenate(outs, axis=0).astype(np.float32)



# revision 2
# speedup vs baseline: 1.5249x; 1.5249x over previous
"""AFM layer kernel for 8 Trainium2 NeuronCores (v8).

Math (per batch b, F=50 fields, E=64, A=10):
  pairs p=(i<j), inter_p = x_i * x_j
  q_c[p]  = inter_p . W[:,c],  logit l_p = sum_c h_c relu(q_c[p] + b_c)
  score = softmax_p(l),  out[b] = sum_p score_p * (inter_p . proj_p)

Reformulation: out[b] = (sum_p e_p r_p) / (sum_p e_p) with e_p = exp(l_p),
r_p = inter_p . proj_p.  Both q_c and r are bilinear forms
x_i^T diag(c) x_j, so inter [B,1225,64] is never materialized.

Engine assignment (per 2-batch iteration; 128 iterations/core; slabs of 8):
  DVE : u[(half,e),(c,j)] = xt * cexp  (moving operand build)
        ers = e * r (PSUM read), fused num/den j-reduce,
        trailing NRELU_DVE c-blocks of the relu
  PE  : Q[(half,i),(c,j)] = st^T @ u   (block-diag stationary, 500 cols)
        r-column matmul (50 cols)
        c-sum l = sum_c s_c relu(q_c) + mask via accumulating
        identity-stationary matmuls (+I for positive-h c-blocks, -I for
        negative, maskt8 with +I) -- sign fold and pair-mask are free
  ACT : relu (PSUM->SBUF, batched over 2 iterations), exp (per 8-slab)
  Pool: optional pre-add of same-sign c-block pairs (SBUF only), shaving
        one accumulating matmul per pair off PE

|h| is folded into W columns on host; columns ordered sign-descending so
+I/-I stationary switches happen once per slab. attention_b is zero in
setup_inputs; a generic bias path (extra K=1 matmul into the Q psum)
activates only when b != 0.
"""

import os
import sys
import numpy as np

for _p in ("/opt/trn_rl_repo",):
    if _p not in sys.path:
        sys.path.insert(0, _p)

B = 2048
NCORES = 8
B_LOC = B // NCORES  # 256
NF = 50
E = 64
NA = 10
NCOL = NA + 1  # 10 W-columns + projection_p
NIT = B_LOC // 2  # 128 two-batch iterations
G = 8  # iterations per slab
NSLAB = NIT // G  # 16
NCHUNK = 16  # input DMA chunks
CH_IT = NIT // NCHUNK  # 8 iterations per chunk
MASK_NEG = -30.0
W5 = NCOL * NF  # 550
WQ = NA * NF  # 500 (W-part of the matmul columns)

# tuning knobs (env-overridable for sweeps; defaults = best known)
NRELU_DVE = int(os.environ.get("KNRELU", "0"))  # trailing c-blocks on DVE
NPOOL_PAIRS = int(os.environ.get("KPOOLPAIRS", "0"))  # c-pairs pre-added on Pool

LAST_RESULTS = None  # stash for test.py (exec_time_ns etc.)


def _build(npos, nneg, has_bias=False, rep=1):
    from contextlib import ExitStack
    import concourse.bass as bass
    import concourse.tile as tile
    from concourse import bacc, mybir

    f32 = mybir.dt.float32
    bf16 = mybir.dt.bfloat16
    AF = mybir.ActivationFunctionType
    OP = mybir.AluOpType

    # same-sign adjacent c-pairs for Pool pre-add, taken from the tail of
    # the negative run, then the tail of the positive run
    pool_pairs = []  # list of (c_lo, c_hi, is_pos)
    if NPOOL_PAIRS > 0:
        cand = []
        c = NA - 1
        while c - 1 >= npos:
            cand.append((c - 1, c, False))
            c -= 2
        c = npos - 1
        while c - 1 >= 0:
            cand.append((c - 1, c, True))
            c -= 2
        pool_pairs = cand[:NPOOL_PAIRS]
    pooled_cs = {c for p in pool_pairs for c in p[:2]}

    nc = bacc.Bacc()
    xt_ext = nc.declare_dram_parameter("xt", [128, NIT * NF], bf16, isOutput=False)
    std_ext = nc.declare_dram_parameter("std", [128, NIT * 100], bf16, isOutput=False)
    cmat_ext = nc.declare_dram_parameter("cmat", [128, NCOL], f32, isOutput=False)
    maskt8_ext = nc.declare_dram_parameter("maskt8", [100, G * NF], bf16, isOutput=False)
    eye_ext = nc.declare_dram_parameter("eye", [100, 200], bf16, isOutput=False)
    onesfin_ext = nc.declare_dram_parameter("onesfin", [100, 2], f32, isOutput=False)
    if has_bias:
        cbias_ext = nc.declare_dram_parameter("cbias", [1, WQ], bf16, isOutput=False)
    out_ext = nc.declare_dram_parameter("out", [B_LOC, 1], f32, isOutput=True)

    with tile.TileContext(nc) as tc, ExitStack() as ctx:
        cpool = ctx.enter_context(tc.tile_pool(name="const", bufs=1))
        xpool = ctx.enter_context(tc.tile_pool(name="xin", bufs=1))
        upool = ctx.enter_context(tc.tile_pool(name="u", bufs=2))
        mpool = ctx.enter_context(tc.tile_pool(name="m", bufs=2))
        epool = ctx.enter_context(tc.tile_pool(name="e", bufs=2))
        smp = ctx.enter_context(tc.tile_pool(name="small", bufs=2))
        accp = ctx.enter_context(tc.tile_pool(name="acc", bufs=1))
        if pool_pairs:
            spool = ctx.enter_context(tc.tile_pool(name="spre", bufs=2))
        psq = ctx.enter_context(tc.tile_pool(name="psq", bufs=2, space="PSUM"))
        psr = ctx.enter_context(tc.tile_pool(name="psr", bufs=2, space="PSUM"))
        psl = ctx.enter_context(tc.tile_pool(name="psl", bufs=2, space="PSUM"))

        # ---- first input chunk before anything else (unblocks compute) ----
        xt_ch = []
        std_ch = []

        def load_chunk(c):
            xc = xpool.tile([128, CH_IT * NF], bf16, name=f"xt{c}")
            nc.sync.dma_start(xc[:], xt_ext[:, c * CH_IT * NF : (c + 1) * CH_IT * NF])
            xt_ch.append(xc)
            sc = xpool.tile([128, CH_IT * 100], bf16, name=f"std{c}")
            nc.sync.dma_start(
                sc[:], std_ext[:, c * CH_IT * 100 : (c + 1) * CH_IT * 100]
            )
            std_ch.append(sc)

        load_chunk(0)

        # ---- constants ----
        cexpS = cpool.tile([128, NCOL], f32)
        nc.gpsimd.dma_start(cexpS[:], cmat_ext[:])
        maskt8 = cpool.tile([100, G * NF], bf16)
        nc.gpsimd.dma_start(maskt8[:], maskt8_ext[:])
        eye = cpool.tile([100, 200], bf16)
        nc.gpsimd.dma_start(eye[:], eye_ext[:])
        onesfin = cpool.tile([100, 2], f32)
        nc.gpsimd.dma_start(onesfin[:], onesfin_ext[:])
        if has_bias:
            cbias = cpool.tile([1, WQ], bf16)
            nc.gpsimd.dma_start(cbias[:], cbias_ext[:])
            ones1 = cpool.tile([1, 100], bf16)
            nc.gpsimd.memset(ones1[:], 1.0)
        eyep = eye[:, 0:100]
        eyen = eye[:, 100:200]

        # ---- remaining input chunks ----
        for c in range(1, NCHUNK):
            load_chunk(c)

        acc = accp.tile([100, 2 * NIT], f32)

        # csum block schedule: mask first (eyep), then positive c's / S's,
        # then negative c's / S's, so the stationary switches once per slab.
        # Entries: ("mask",), ("c", c, is_pos), ("s", pair_idx, is_pos).
        csum_blocks = [("mask", 0, True)]
        for c in range(npos):
            if c not in pooled_cs:
                csum_blocks.append(("c", c, True))
        for i, p in enumerate(pool_pairs):
            if p[2]:
                csum_blocks.append(("s", i, True))
        for c in range(npos, NA):
            if c not in pooled_cs:
                csum_blocks.append(("c", c, False))
        for i, p in enumerate(pool_pairs):
            if not p[2]:
                csum_blocks.append(("s", i, False))
        nblk = len(csum_blocks)
        # split into 4 parts, sizes as even as possible, larger parts first
        q_, r_ = divmod(nblk, 4)
        part_sizes = [q_ + (1 if i < r_ else 0) for i in range(4)]
        csum_groups = []
        pos = 0
        for sz in part_sizes:
            csum_groups.append(csum_blocks[pos : pos + sz])
            pos += sz

        # state carried between slabs for the 1-slab-delayed tail
        prev = None  # (m_sl, r_ps, s_data, ell_done, s_tiles)

        def pair_csum(ell, m_sl, k0):
            """csum for one iteration pair (k0, k0+1) into its own
            accumulation-group region of ell (final slab: avoids the serial
            end-of-kernel csum burst). Pure PE path (no Pool pre-adds)."""
            reg = ell[:, k0 * NF : (k0 + 2) * NF]
            nc.tensor.matmul(
                reg, eyep, maskt8[:, k0 * NF : (k0 + 2) * NF],
                start=True, stop=False,
            )
            for c in range(NA):
                ey = eyep if c < npos else eyen
                rhs = m_sl[:].rearrange("p (g cj) -> p g cj", cj=WQ)[
                    :, k0 : k0 + 2, c * NF : (c + 1) * NF
                ]
                nc.tensor.matmul(reg, ey, rhs, start=False, stop=(c == NA - 1))

        def csum_mms(m_sl, s_tiles, part):
            """Emit csum group `part` (0..3) for slab with relu'd m_sl.

            Accumulating matmuls (mask + signed identity c-sums + Pool
            pre-added pair blocks) split into 4 groups so the PE burst
            interleaves with the next slab's Q matmuls.
            """
            ell = csum_mms.ell
            group = csum_groups[part]
            for gi, (kind, idx, is_pos) in enumerate(group):
                first = part == 0 and gi == 0
                last = part == 3 and gi == len(group) - 1
                ey = eyep if is_pos else eyen
                if kind == "mask":
                    rhs = maskt8[:]
                elif kind == "c":
                    rhs = m_sl[:].rearrange("p (g cj) -> p g cj", cj=WQ)[
                        :, :, idx * NF : (idx + 1) * NF
                    ]
                else:  # "s": Pool pre-added pair
                    rhs = s_tiles[idx][:]
                nc.tensor.matmul(ell[:], ey, rhs, start=first, stop=last)

        def emit_pool_preadds(m_sl):
            """Pool (gpsimd) pre-adds same-sign c-block pairs of m_sl into
            S tiles (SBUF->SBUF), each replacing two PE csum matmuls with
            one."""
            s_tiles = []
            for i, (ca, cb, _pos) in enumerate(pool_pairs):
                st = spool.tile([100, G * NF], bf16, tag=f"sp{i}")
                mv = m_sl[:].rearrange("p (g cj) -> p g cj", cj=WQ)
                nc.gpsimd.tensor_tensor(
                    st[:].rearrange("p (g j) -> p g j", j=NF),
                    mv[:, :, ca * NF : (ca + 1) * NF],
                    mv[:, :, cb * NF : (cb + 1) * NF],
                    op=OP.add,
                )
                s_tiles.append(st)
            return s_tiles

        def emit_tail_rest(r_ps, s_data, ell_done=None):
            """exp + ers + fused num/den j-reduce for slab s_data."""
            ell = ell_done if ell_done is not None else csum_mms.ell
            ec = epool.tile([100, 2 * G * NF], bf16, tag="ecomb")
            e_v = ec[:, G * NF : 2 * G * NF]
            nc.scalar.activation(e_v, ell[:], AF.Exp)
            nc.vector.tensor_tensor(ec[:, 0 : G * NF], e_v, r_ps[:], op=OP.mult)
            base = s_data * G
            acc_v = acc[:].rearrange("p (two nit) -> p two nit", nit=NIT)[
                :, :, base : base + G
            ]
            nc.vector.reduce_sum(
                acc_v,
                ec[:].rearrange("p (two g j) -> p two g j", g=G, j=NF),
                axis=mybir.AxisListType.X,
            )

        for rs in range(rep * NSLAB + 1):
            last = rs == rep * NSLAB
            s = rs % NSLAB
            ch = s // (NSLAB // NCHUNK)
            xt_t = xt_ch[ch]
            std_t = std_ch[ch]
            it0 = (s % (NSLAB // NCHUNK)) * G * NF  # xt col offset in chunk
            st0 = (s % (NSLAB // NCHUNK)) * G * 100

            if not last:
                m_sl = mpool.tile([100, G * WQ], bf16, tag="msl")
                r_ps = psr.tile([100, G * NF], f32, tag="rps")
                u_sl = upool.tile([128, G * W5], bf16, tag="usl")
                # u[(half,e), (it,c,j)] = xt[(half,e), (it,j)] * Cmat[(half,e), c]
                # First slab: emit the first pair's u separately so the first
                # Q matmul (and the ACT pipeline behind it) unblocks early.
                u_groups = ((0, 2), (2, G)) if rs == 0 else ((0, G),)
                for g0, g1 in u_groups:
                    for c in range(NCOL):
                        nc.vector.tensor_scalar(
                            u_sl[:]
                            .rearrange("p (g cj) -> p g cj", cj=W5)[
                                :, g0:g1, c * NF : (c + 1) * NF
                            ],
                            xt_t[:, it0 + g0 * NF : it0 + g1 * NF].rearrange(
                                "p (g j) -> p g j", j=NF
                            ),
                            cexpS[:, c : c + 1],
                            None,
                            op0=OP.mult,
                        )
            if prev is not None and rs > 0 and prev[3] is None:
                csum_mms.ell = psl.tile([100, G * NF], f32, tag="ell")
            final_slab = rs == rep * NSLAB - 1
            final_ell = None
            q_t = None
            for k in range(G):
                if not last:
                    soff = st0 + k * 100
                    uoff = k * W5
                    if k % 2 == 0:
                        q_t = psq.tile([100, 1024], f32, tag="qt")
                    qoff = (k % 2) * 512
                    st_ap = std_t[:, soff : soff + 100]
                    if has_bias:
                        nc.tensor.matmul(
                            q_t[:, qoff : qoff + WQ], ones1[:], cbias[:],
                            start=True, stop=False,
                        )
                    nc.tensor.matmul(
                        q_t[:, qoff : qoff + WQ], st_ap, u_sl[:, uoff : uoff + WQ],
                        start=not has_bias, stop=True,
                    )
                    nc.tensor.matmul(
                        r_ps[:, k * NF : (k + 1) * NF], st_ap,
                        u_sl[:, uoff + WQ : uoff + W5],
                        start=True, stop=True,
                    )
                if prev is not None and k % 2 == 1 and prev[3] is None:
                    csum_mms(prev[0], prev[4], k // 2)
                if final_slab and k % 2 == 1 and k > 1:
                    # final slab: csum the PREVIOUS pair now (its relu is
                    # done); last pair + exp handled after the loop
                    if final_ell is None:
                        final_ell = psl.tile([100, G * NF], f32, tag="ell")
                    pair_csum(final_ell, m_sl, k - 3)
                if not last and k % 2 == 1:
                    # relu two iterations: q_t[:, {0,512}+0:500] -> m_sl.
                    # Leading c-blocks on ACT, trailing NRELU_DVE blocks on DVE
                    # to balance engine load.
                    split = (NA - NRELU_DVE) * NF
                    qv = q_t[:].rearrange("p (two x) -> p two x", two=2)
                    mv = m_sl[:].rearrange("p (g cj) -> p g cj", cj=WQ)[
                        :, k - 1 : k + 1, :
                    ]
                    if split > 0:
                        nc.scalar.activation(
                            mv[:, :, 0:split], qv[:, :, 0:split], AF.Relu
                        )
                    if NRELU_DVE:
                        nc.vector.tensor_scalar(
                            mv[:, :, split:WQ],
                            qv[:, :, split:WQ],
                            0.0,
                            None,
                            op0=OP.max,
                        )
            if final_slab:
                pair_csum(final_ell, m_sl, G - 2)
            if prev is not None:
                emit_tail_rest(prev[1], prev[2], prev[3])
            if not last:
                s_tiles = (
                    emit_pool_preadds(m_sl)
                    if (pool_pairs and not final_slab)
                    else None
                )
                prev = (m_sl, r_ps, s, final_ell, s_tiles)

        # final: per-batch partition sums (num, den), divide, store.
        # The [2, 256] result lives in a spare rotation of the ell tag to
        # stay within the 8-bank PSUM budget.
        pft = psl.tile([100, G * NF], f32, tag="ell")
        pf = pft[0:2, 0 : 2 * NIT]
        nc.tensor.matmul(pf, onesfin[:], acc[:], start=True, stop=True)
        rcp = smp.tile([2, NIT], f32, tag="rcp")
        nc.vector.reciprocal(rcp[:], pf[:, NIT : 2 * NIT])
        res = smp.tile([2, NIT], f32, tag="res")
        nc.vector.tensor_tensor(res[:], pf[:, 0:NIT], rcp[:], op=OP.mult)
        out_v = out_ext[:].rearrange("(i m) o -> m (i o)", m=2)
        nc.sync.dma_start(out_v, res[:])

    nc.compile()
    return nc


def _host_prep(x, attention_W, attention_b, projection_h, projection_p):
    import ml_dtypes

    bf = ml_dtypes.bfloat16
    x = np.ascontiguousarray(np.asarray(x, dtype=np.float32))
    W = np.asarray(attention_W, dtype=np.float32)
    bv = np.asarray(attention_b, dtype=np.float32)
    h = np.asarray(projection_h, dtype=np.float32).reshape(-1)
    p = np.asarray(projection_p, dtype=np.float32).reshape(-1)

    habs = np.abs(h)
    sgn = np.sign(h)
    order = np.argsort(-sgn, kind="stable")
    Wp = (W * habs[None, :])[:, order]
    bp = (bv * habs)[order]
    npos = int((sgn[order] > 0).sum())
    has_bias = bool(np.any(bv != 0.0))

    Cmat = np.concatenate([Wp, p[:, None]], axis=1)  # [64, 11]
    cmat = np.ascontiguousarray(
        np.concatenate([Cmat, Cmat], axis=0), dtype=np.float32
    )  # [128, 11]
    cbias = np.repeat(bp, NF).reshape(1, WQ).astype(bf)  # [1,500]
    im = np.arange(NF)
    m50 = np.where(im[:, None] < im[None, :], 0.0, MASK_NEG).astype(np.float32)
    maskt = np.concatenate([m50, m50], axis=0)  # [100, 50]
    maskt8 = np.ascontiguousarray(np.tile(maskt, (1, G))).astype(bf)  # [100,400]
    eye = np.zeros((100, 200), dtype=np.float32)
    eye[:, 0:100] = np.eye(100)
    eye[:, 100:200] = -np.eye(100)
    eye = eye.astype(bf)
    onesfin = np.zeros((100, 2), dtype=np.float32)
    onesfin[0:50, 0] = 1.0
    onesfin[50:100, 1] = 1.0

    # per-core packed layouts
    xcores = x.reshape(NCORES, B_LOC, NF, E)
    xt_bf = np.transpose(xcores, (0, 1, 3, 2)).astype(bf)  # [C, 256, 64, 50]
    xt_it = xt_bf.reshape(NCORES, NIT, 2, E, NF)  # [C, it, half, e, j]
    # xt [128, NIT*50]: rows (half, e), cols (it, j)
    xt_all = np.ascontiguousarray(
        xt_it.transpose(0, 2, 3, 1, 4).reshape(NCORES, 128, NIT * NF)
    )
    # std [128, NIT*100]: block-diag per iteration
    std_all = np.zeros((NCORES, 2, E, NIT, 2, NF), dtype=bf)  # [C,rh,e,it,ch,j]
    std_all[:, 0, :, :, 0, :] = xt_it[:, :, 0].transpose(0, 2, 1, 3)
    std_all[:, 1, :, :, 1, :] = xt_it[:, :, 1].transpose(0, 2, 1, 3)
    std_all = np.ascontiguousarray(std_all.reshape(NCORES, 128, NIT * 100))

    return npos, has_bias, xt_all, std_all, cmat, cbias, maskt8, eye, onesfin


def kernel(x, attention_W, attention_b, projection_h, projection_p):
    global LAST_RESULTS
    from concourse.bass_utils import run_bass_kernel_spmd

    npos, has_bias, xt_all, std_all, cmat, cbias, maskt8, eye, onesfin = _host_prep(
        x, attention_W, attention_b, projection_h, projection_p
    )
    nc = _build(npos, NA - npos, has_bias=has_bias)

    in_maps = []
    for c in range(NCORES):
        m = {
            "xt": xt_all[c],
            "std": std_all[c],
            "cmat": cmat,
            "maskt8": maskt8,
            "eye": eye,
            "onesfin": onesfin,
        }
        if has_bias:
            m["cbias"] = cbias
        in_maps.append(m)
    trace = os.environ.get("BASS_KERNEL_TRACE", "0") == "1"
    res = run_bass_kernel_spmd(nc, in_maps, core_ids=list(range(NCORES)), trace=trace)
    LAST_RESULTS = res
    outs = [np.asarray(r["out"]).reshape(B_LOC, 1) for r in res.results]
    return np.concatenate(outs, axis=0).astype(np.float32)


# revision 19
# speedup vs baseline: 1.8213x; 1.1943x over previous
"""AFM layer kernel for 8 Trainium2 NeuronCores (v8).

Math (per batch b, F=50 fields, E=64, A=10):
  pairs p=(i<j), inter_p = x_i * x_j
  q_c[p]  = inter_p . W[:,c],  logit l_p = sum_c h_c relu(q_c[p] + b_c)
  score = softmax_p(l),  out[b] = sum_p score_p * (inter_p . proj_p)

Reformulation: out[b] = (sum_p e_p r_p) / (sum_p e_p) with e_p = exp(l_p),
r_p = inter_p . proj_p.  Both q_c and r are bilinear forms
x_i^T diag(c) x_j, so inter [B,1225,64] is never materialized.

Engine assignment (per 2-batch iteration; 128 iterations/core; slabs of 8):
  DVE : u[(half,e),(c,j)] = xt * cexp  (moving operand build)
        ers = e * r (PSUM read), fused num/den j-reduce,
        trailing NRELU_DVE c-blocks of the relu
  PE  : Q[(half,i),(c,j)] = st^T @ u   (block-diag stationary, 500 cols)
        r-column matmul (50 cols)
        c-sum l = sum_c s_c relu(q_c) + mask via accumulating
        identity-stationary matmuls (+I for positive-h c-blocks, -I for
        negative, maskt8 with +I) -- sign fold and pair-mask are free
  ACT : relu (PSUM->SBUF, batched over 2 iterations), exp (per 8-slab)
  Pool: optional pre-add of same-sign c-block pairs (SBUF only), shaving
        one accumulating matmul per pair off PE

|h| is folded into W columns on host; columns ordered sign-descending so
+I/-I stationary switches happen once per slab. attention_b is zero in
setup_inputs; a generic bias path (extra K=1 matmul into the Q psum)
activates only when b != 0.
"""

import os
import sys
import numpy as np

for _p in ("/opt/trn_rl_repo",):
    if _p not in sys.path:
        sys.path.insert(0, _p)

B = 2048
NCORES = 8
B_LOC = B // NCORES  # 256
NF = 50
E = 64
NA = 10
NCOL = NA + 1  # 10 W-columns + projection_p
NIT = B_LOC // 2  # 128 two-batch iterations
G = 8  # iterations per slab
NSLAB = NIT // G  # 16
NCHUNK = 16  # input DMA chunks
CH_IT = NIT // NCHUNK  # 8 iterations per chunk
MASK_NEG = -30.0
W5 = NCOL * NF  # 550
WQ = NA * NF  # 500 (W-part of the matmul columns)

# tuning knobs (env-overridable for sweeps; defaults = best known)
NRELU_DVE = int(os.environ.get("KNRELU", "0"))  # trailing c-blocks on DVE
NPOOL_PAIRS = int(os.environ.get("KPOOLPAIRS", "0"))  # c-pairs pre-added on Pool
DVESUF = int(os.environ.get("KDVESUF", "0"))  # extra DVE relu cols, last pair only
CSUMF8 = int(os.environ.get("KCSUMF8", "0"))  # fp8 m + DoubleRow pair csum
# pair indices (odd k: 1,3,5,7) whose relu runs fully on DVE as one big instr
DVEG = tuple(
    int(t) for t in os.environ.get("KDVEG", "").split(",") if t.strip()
)

LAST_RESULTS = None  # stash for test.py (exec_time_ns etc.)


def _build(npos, nneg, has_bias=False, rep=1):
    from contextlib import ExitStack
    import concourse.bass as bass
    import concourse.tile as tile
    from concourse import bacc, mybir

    f32 = mybir.dt.float32
    bf16 = mybir.dt.bfloat16
    AF = mybir.ActivationFunctionType
    OP = mybir.AluOpType

    # same-sign adjacent c-pairs for Pool pre-add, taken from the tail of
    # the negative run, then the tail of the positive run. c_lo == -1 means
    # the pair-mask constant block (positive sign, pairs with c0).
    pool_pairs = []  # list of (c_lo, c_hi, is_pos)
    pool_mask = int(os.environ.get("KPOOLMASK", "0")) and npos >= 1 and not CSUMF8
    if (NPOOL_PAIRS > 0 and not CSUMF8) or pool_mask:
        cand = []
        if pool_mask:
            cand.append((-1, 0, True))
        c = NA - 1
        while c - 1 >= npos:
            cand.append((c - 1, c, False))
            c -= 2
        c = npos - 1
        lo = 1 if pool_mask else 0
        while c - 1 >= lo:
            cand.append((c - 1, c, True))
            c -= 2
        pool_pairs = cand[: NPOOL_PAIRS + (1 if pool_mask else 0)]
    pooled_cs = {c for p in pool_pairs for c in p[:2] if c >= 0}
    mask_pooled = any(p[0] == -1 for p in pool_pairs)

    f8 = mybir.dt.float8e4
    m_dt = f8 if CSUMF8 else bf16

    nc = bacc.Bacc()
    xt_ext = nc.declare_dram_parameter("xt", [128, NIT * NF], bf16, isOutput=False)
    std_ext = nc.declare_dram_parameter("std", [128, NIT * 100], bf16, isOutput=False)
    cmat_ext = nc.declare_dram_parameter("cmat", [128, NCOL], f32, isOutput=False)
    maskt8_ext = nc.declare_dram_parameter("maskt8", [100, G * NF], bf16, isOutput=False)
    eye_ext = nc.declare_dram_parameter("eye", [100, 200], bf16, isOutput=False)
    if CSUMF8:
        # fp8 identity pairs for DoubleRow csum: (kt0,kt1) sign combos
        # (+,+) cols 0:200, (+,-) 200:400, (-,-) 400:600
        eye8_ext = nc.declare_dram_parameter("eye8", [100, 600], f8, isOutput=False)
    onesfin_ext = nc.declare_dram_parameter("onesfin", [100, 2], f32, isOutput=False)
    if has_bias:
        cbias_ext = nc.declare_dram_parameter("cbias", [1, WQ], bf16, isOutput=False)
    out_ext = nc.declare_dram_parameter("out", [B_LOC, 1], f32, isOutput=True)

    with tile.TileContext(nc) as tc, ExitStack() as ctx:
        cpool = ctx.enter_context(tc.tile_pool(name="const", bufs=1))
        xpool = ctx.enter_context(tc.tile_pool(name="xin", bufs=1))
        upool = ctx.enter_context(tc.tile_pool(name="u", bufs=2))
        mpool = ctx.enter_context(tc.tile_pool(name="m", bufs=2))
        epool = ctx.enter_context(tc.tile_pool(name="e", bufs=2))
        smp = ctx.enter_context(tc.tile_pool(name="small", bufs=2))
        accp = ctx.enter_context(tc.tile_pool(name="acc", bufs=1))
        if pool_pairs:
            spool = ctx.enter_context(tc.tile_pool(name="spre", bufs=2))
        psq = ctx.enter_context(tc.tile_pool(name="psq", bufs=2, space="PSUM"))
        psr = ctx.enter_context(tc.tile_pool(name="psr", bufs=2, space="PSUM"))
        psl = ctx.enter_context(tc.tile_pool(name="psl", bufs=2, space="PSUM"))

        # ---- first input chunk before anything else (unblocks compute) ----
        xt_ch = []
        std_ch = []

        def load_chunk(c):
            xc = xpool.tile([128, CH_IT * NF], bf16, name=f"xt{c}")
            nc.sync.dma_start(xc[:], xt_ext[:, c * CH_IT * NF : (c + 1) * CH_IT * NF])
            xt_ch.append(xc)
            sc = xpool.tile([128, CH_IT * 100], bf16, name=f"std{c}")
            nc.sync.dma_start(
                sc[:], std_ext[:, c * CH_IT * 100 : (c + 1) * CH_IT * 100]
            )
            std_ch.append(sc)

        load_chunk(0)

        # ---- constants ----
        cexpS = cpool.tile([128, NCOL], f32)
        nc.gpsimd.dma_start(cexpS[:], cmat_ext[:])
        maskt8 = cpool.tile([100, G * NF], bf16)
        nc.gpsimd.dma_start(maskt8[:], maskt8_ext[:])
        eye = cpool.tile([100, 200], bf16)
        nc.gpsimd.dma_start(eye[:], eye_ext[:])
        if CSUMF8:
            eye8 = cpool.tile([100, 600], f8)
            nc.gpsimd.dma_start(eye8[:], eye8_ext[:])
        onesfin = cpool.tile([100, 2], f32)
        nc.gpsimd.dma_start(onesfin[:], onesfin_ext[:])
        if has_bias:
            cbias = cpool.tile([1, WQ], bf16)
            nc.gpsimd.dma_start(cbias[:], cbias_ext[:])
            ones1 = cpool.tile([1, 100], bf16)
            nc.gpsimd.memset(ones1[:], 1.0)
        eyep = eye[:, 0:100]
        eyen = eye[:, 100:200]

        # ---- remaining input chunks ----
        for c in range(1, NCHUNK):
            load_chunk(c)

        acc = accp.tile([100, 2 * NIT], f32)

        # csum block schedule: mask first (eyep), then positive c's / S's,
        # then negative c's / S's, so the stationary switches once per slab.
        # Entries: ("mask",), ("c", c, is_pos), ("s", pair_idx, is_pos).
        if CSUMF8:
            # consecutive c-pairs; the c-pair sum is folded into the 2x-deep
            # DoubleRow contraction, signs live in the eye8 ktile slices:
            # combo 0=(+,+), 1=(+,-), 2=(-,-)
            csum_blocks = [("mask", 0, True)]
            for a in range(0, NA, 2):
                sa, sb = a < npos, a + 1 < npos
                combo = 0 if (sa and sb) else (1 if sa else 2)
                csum_blocks.append(("pair", a, combo))
        else:
            csum_blocks = [] if mask_pooled else [("mask", 0, True)]
            for c in range(npos):
                if c not in pooled_cs:
                    csum_blocks.append(("c", c, True))
            for i, p in enumerate(pool_pairs):
                if p[2]:
                    csum_blocks.append(("s", i, True))
            for c in range(npos, NA):
                if c not in pooled_cs:
                    csum_blocks.append(("c", c, False))
            for i, p in enumerate(pool_pairs):
                if not p[2]:
                    csum_blocks.append(("s", i, False))
        nblk = len(csum_blocks)
        # split into 4 parts, sizes as even as possible, larger parts first
        q_, r_ = divmod(nblk, 4)
        part_sizes = [q_ + (1 if i < r_ else 0) for i in range(4)]
        csum_groups = []
        pos = 0
        for sz in part_sizes:
            csum_groups.append(csum_blocks[pos : pos + sz])
            pos += sz

        # state carried between slabs for the 1-slab-delayed tail
        prev = None  # (m_sl, r_ps, s_data, ell_done, s_tiles)

        GNF = G * NF  # one c-block of m: [100, (g, j)] contiguous

        def pair_csum(ell, m_sl, k0):
            """csum for one iteration pair (k0, k0+1) into its own
            accumulation-group region of ell (final slab: avoids the serial
            end-of-kernel csum burst). Plain per-c matmuls."""
            reg = ell[:, k0 * NF : (k0 + 2) * NF]
            nc.tensor.matmul(
                reg, eyep, maskt8[:, k0 * NF : (k0 + 2) * NF],
                start=True, stop=False,
            )
            mv = m_sl[:].rearrange("p (c g j) -> p c g j", g=G, j=NF)
            for c in range(NA):
                if CSUMF8:
                    ey = eye8[:, 0:100] if c < npos else eye8[:, 400:500]
                else:
                    ey = eyep if c < npos else eyen
                rhs = mv[:, c, k0 : k0 + 2, :]
                nc.tensor.matmul(reg, ey, rhs, start=False, stop=(c == NA - 1))

        def csum_mms(m_sl, s_tiles, part):
            """Emit csum group `part` (0..3) for slab with relu'd m_sl.

            Accumulating matmuls (mask + signed identity c-sums / fp8
            DoubleRow c-pair sums / Pool pre-added pair blocks) split into
            4 groups so the PE burst interleaves with the next slab's Q
            matmuls.
            """
            ell = csum_mms.ell
            group = csum_groups[part]
            for gi, (kind, idx, third) in enumerate(group):
                first = part == 0 and gi == 0
                last = part == 3 and gi == len(group) - 1
                if kind == "mask":
                    nc.tensor.matmul(
                        ell[:], eyep, maskt8[:], start=first, stop=last
                    )
                elif kind == "pair":
                    lhsT = eye8[:, third * 200 : third * 200 + 200].rearrange(
                        "p (kt m) -> p kt m", kt=2
                    )
                    rhs = m_sl[:, idx * GNF : (idx + 2) * GNF].rearrange(
                        "p (kt n) -> p kt n", kt=2
                    )
                    nc.tensor.matmul(
                        ell[:], lhsT, rhs, start=first, stop=last,
                        perf_mode=mybir.MatmulPerfMode.DoubleRow,
                    )
                else:
                    ey = eyep if third else eyen
                    if kind == "c":
                        rhs = m_sl[:, idx * GNF : (idx + 1) * GNF]
                    else:  # "s": Pool pre-added pair
                        rhs = s_tiles[idx][:]
                    nc.tensor.matmul(ell[:], ey, rhs, start=first, stop=last)

        def emit_pool_preadds(m_sl):
            """Pool (gpsimd) pre-adds same-sign c-block pairs of m_sl into
            S tiles (SBUF->SBUF), each replacing two PE csum matmuls with
            one."""
            s_tiles = []
            for i, (ca, cb, _pos) in enumerate(pool_pairs):
                st = spool.tile([100, GNF], bf16, tag=f"sp{i}")
                in0 = maskt8[:] if ca < 0 else m_sl[:, ca * GNF : (ca + 1) * GNF]
                nc.gpsimd.tensor_tensor(
                    st[:], in0, m_sl[:, cb * GNF : (cb + 1) * GNF], op=OP.add
                )
                s_tiles.append(st)
            return s_tiles

        def emit_tail_rest(r_ps, s_data, ell_done=None):
            """exp + ers + fused num/den j-reduce for slab s_data."""
            ell = ell_done if ell_done is not None else csum_mms.ell
            ec = epool.tile([100, 2 * G * NF], bf16, tag="ecomb")
            e_v = ec[:, G * NF : 2 * G * NF]
            nc.scalar.activation(e_v, ell[:], AF.Exp)
            nc.vector.tensor_tensor(ec[:, 0 : G * NF], e_v, r_ps[:], op=OP.mult)
            base = s_data * G
            acc_v = acc[:].rearrange("p (two nit) -> p two nit", nit=NIT)[
                :, :, base : base + G
            ]
            nc.vector.reduce_sum(
                acc_v,
                ec[:].rearrange("p (two g j) -> p two g j", g=G, j=NF),
                axis=mybir.AxisListType.X,
            )

        for rs in range(rep * NSLAB + 1):
            last = rs == rep * NSLAB
            s = rs % NSLAB
            ch = s // (NSLAB // NCHUNK)
            xt_t = xt_ch[ch]
            std_t = std_ch[ch]
            it0 = (s % (NSLAB // NCHUNK)) * G * NF  # xt col offset in chunk
            st0 = (s % (NSLAB // NCHUNK)) * G * 100

            if not last:
                m_sl = mpool.tile([100, G * WQ], m_dt, tag="msl")
                r_ps = psr.tile([100, G * NF], f32, tag="rps")
                u_sl = upool.tile([128, G * W5], bf16, tag="usl")
                # u[(half,e), (it,c,j)] = xt[(half,e), (it,j)] * Cmat[(half,e), c]
                # First slab: emit the first pair's u separately so the first
                # Q matmul (and the ACT pipeline behind it) unblocks early.
                u_groups = ((0, 2), (2, G)) if rs == 0 else ((0, G),)
                for g0, g1 in u_groups:
                    for c in range(NCOL):
                        nc.vector.tensor_scalar(
                            u_sl[:]
                            .rearrange("p (g cj) -> p g cj", cj=W5)[
                                :, g0:g1, c * NF : (c + 1) * NF
                            ],
                            xt_t[:, it0 + g0 * NF : it0 + g1 * NF].rearrange(
                                "p (g j) -> p g j", j=NF
                            ),
                            cexpS[:, c : c + 1],
                            None,
                            op0=OP.mult,
                        )
            if prev is not None and rs > 0 and prev[3] is None:
                csum_mms.ell = psl.tile([100, G * NF], f32, tag="ell")
            final_slab = rs == rep * NSLAB - 1
            final_ell = None
            q_t = None
            for k in range(G):
                if not last:
                    soff = st0 + k * 100
                    uoff = k * W5
                    if k % 2 == 0:
                        q_t = psq.tile([100, 1024], f32, tag="qt")
                    qoff = (k % 2) * 512
                    st_ap = std_t[:, soff : soff + 100]
                    if has_bias:
                        nc.tensor.matmul(
                            q_t[:, qoff : qoff + WQ], ones1[:], cbias[:],
                            start=True, stop=False,
                        )
                    nc.tensor.matmul(
                        q_t[:, qoff : qoff + WQ], st_ap, u_sl[:, uoff : uoff + WQ],
                        start=not has_bias, stop=True,
                    )
                    nc.tensor.matmul(
                        r_ps[:, k * NF : (k + 1) * NF], st_ap,
                        u_sl[:, uoff + WQ : uoff + W5],
                        start=True, stop=True,
                    )
                if prev is not None and k % 2 == 1 and prev[3] is None:
                    csum_mms(prev[0], prev[4], k // 2)
                if final_slab and k % 2 == 1 and k > 1:
                    # final slab: csum the PREVIOUS pair now (its relu is
                    # done); last pair + exp handled after the loop
                    if final_ell is None:
                        final_ell = psl.tile([100, G * NF], f32, tag="ell")
                    pair_csum(final_ell, m_sl, k - 3)
                if not last and k % 2 == 1:
                    # relu two iterations: q_t[:, {0,512}+0:500] -> m_sl.
                    # Leading columns on ACT, trailing columns on DVE to
                    # balance engine load: NRELU_DVE c-blocks on every pair,
                    # plus DVESUF extra columns on the slab's last pair only
                    # (fine-grained split amortizing DVE's fixed PSUM-access
                    # cost over one larger instruction).
                    split = (NA - NRELU_DVE) * NF
                    if k == G - 1:
                        split = min(split, WQ - (DVESUF // NF) * NF)
                    if k in DVEG:
                        split = 0  # whole pair on DVE (one big instruction)
                    csp = split // NF  # c-block boundary (NF-aligned split)
                    qv = q_t[:].rearrange("p (two x) -> p two x", two=2)[
                        :, :, 0:WQ
                    ].rearrange("p two (c j) -> p two c j", j=NF)
                    mv = m_sl[:].rearrange("p (c g j) -> p g c j", g=G, j=NF)[
                        :, k - 1 : k + 1, :, :
                    ]
                    if csp > 0:
                        nc.scalar.activation(
                            mv[:, :, 0:csp, :], qv[:, :, 0:csp, :], AF.Relu
                        )
                    if csp < NA:
                        nc.vector.tensor_scalar(
                            mv[:, :, csp:NA, :],
                            qv[:, :, csp:NA, :],
                            0.0,
                            None,
                            op0=OP.max,
                        )
            if final_slab:
                pair_csum(final_ell, m_sl, G - 2)
            if prev is not None:
                emit_tail_rest(prev[1], prev[2], prev[3])
            if not last:
                s_tiles = (
                    emit_pool_preadds(m_sl)
                    if (pool_pairs and not final_slab)
                    else None
                )
                prev = (m_sl, r_ps, s, final_ell, s_tiles)

        # final: per-batch partition sums (num, den), divide, store.
        # The [2, 256] result lives in a spare rotation of the ell tag to
        # stay within the 8-bank PSUM budget.
        pft = psl.tile([100, G * NF], f32, tag="ell")
        pf = pft[0:2, 0 : 2 * NIT]
        nc.tensor.matmul(pf, onesfin[:], acc[:], start=True, stop=True)
        rcp = smp.tile([2, NIT], f32, tag="rcp")
        nc.vector.reciprocal(rcp[:], pf[:, NIT : 2 * NIT])
        res = smp.tile([2, NIT], f32, tag="res")
        nc.vector.tensor_tensor(res[:], pf[:, 0:NIT], rcp[:], op=OP.mult)
        out_v = out_ext[:].rearrange("(i m) o -> m (i o)", m=2)
        nc.sync.dma_start(out_v, res[:])

    nc.compile()
    return nc


def _host_prep(x, attention_W, attention_b, projection_h, projection_p):
    import ml_dtypes

    bf = ml_dtypes.bfloat16
    x = np.ascontiguousarray(np.asarray(x, dtype=np.float32))
    W = np.asarray(attention_W, dtype=np.float32)
    bv = np.asarray(attention_b, dtype=np.float32)
    h = np.asarray(projection_h, dtype=np.float32).reshape(-1)
    p = np.asarray(projection_p, dtype=np.float32).reshape(-1)

    habs = np.abs(h)
    sgn = np.sign(h)
    order = np.argsort(-sgn, kind="stable")
    Wp = (W * habs[None, :])[:, order]
    bp = (bv * habs)[order]
    npos = int((sgn[order] > 0).sum())
    has_bias = bool(np.any(bv != 0.0))

    Cmat = np.concatenate([Wp, p[:, None]], axis=1)  # [64, 11]
    cmat = np.ascontiguousarray(
        np.concatenate([Cmat, Cmat], axis=0), dtype=np.float32
    )  # [128, 11]
    cbias = np.repeat(bp, NF).reshape(1, WQ).astype(bf)  # [1,500]
    im = np.arange(NF)
    m50 = np.where(im[:, None] < im[None, :], 0.0, MASK_NEG).astype(np.float32)
    maskt = np.concatenate([m50, m50], axis=0)  # [100, 50]
    maskt8 = np.ascontiguousarray(np.tile(maskt, (1, G))).astype(bf)  # [100,400]
    eye = np.zeros((100, 200), dtype=np.float32)
    eye[:, 0:100] = np.eye(100)
    eye[:, 100:200] = -np.eye(100)
    eye = eye.astype(bf)
    onesfin = np.zeros((100, 2), dtype=np.float32)
    onesfin[0:50, 0] = 1.0
    onesfin[50:100, 1] = 1.0

    from concourse import mybir as _mb

    f8np = _mb.dt.np(_mb.dt.float8e4)
    e100 = np.eye(100, dtype=np.float32)
    eye8 = np.zeros((100, 600), dtype=np.float32)
    eye8[:, 0:100] = e100
    eye8[:, 100:200] = e100  # (+,+)
    eye8[:, 200:300] = e100
    eye8[:, 300:400] = -e100  # (+,-)
    eye8[:, 400:500] = -e100
    eye8[:, 500:600] = -e100  # (-,-)
    eye8 = np.ascontiguousarray(eye8.astype(f8np))

    # per-core packed layouts
    xcores = x.reshape(NCORES, B_LOC, NF, E)
    xt_bf = np.transpose(xcores, (0, 1, 3, 2)).astype(bf)  # [C, 256, 64, 50]
    xt_it = xt_bf.reshape(NCORES, NIT, 2, E, NF)  # [C, it, half, e, j]
    # xt [128, NIT*50]: rows (half, e), cols (it, j)
    xt_all = np.ascontiguousarray(
        xt_it.transpose(0, 2, 3, 1, 4).reshape(NCORES, 128, NIT * NF)
    )
    # std [128, NIT*100]: block-diag per iteration
    std_all = np.zeros((NCORES, 2, E, NIT, 2, NF), dtype=bf)  # [C,rh,e,it,ch,j]
    std_all[:, 0, :, :, 0, :] = xt_it[:, :, 0].transpose(0, 2, 1, 3)
    std_all[:, 1, :, :, 1, :] = xt_it[:, :, 1].transpose(0, 2, 1, 3)
    std_all = np.ascontiguousarray(std_all.reshape(NCORES, 128, NIT * 100))

    return npos, has_bias, xt_all, std_all, cmat, cbias, maskt8, eye, onesfin, eye8


def kernel(x, attention_W, attention_b, projection_h, projection_p):
    global LAST_RESULTS
    from concourse.bass_utils import run_bass_kernel_spmd

    npos, has_bias, xt_all, std_all, cmat, cbias, maskt8, eye, onesfin, eye8 = (
        _host_prep(x, attention_W, attention_b, projection_h, projection_p)
    )
    nc = _build(npos, NA - npos, has_bias=has_bias)

    in_maps = []
    for c in range(NCORES):
        m = {
            "xt": xt_all[c],
            "std": std_all[c],
            "cmat": cmat,
            "maskt8": maskt8,
            "eye": eye,
            "onesfin": onesfin,
        }
        if CSUMF8:
            m["eye8"] = eye8
        if has_bias:
            m["cbias"] = cbias
        in_maps.append(m)
    trace = os.environ.get("BASS_KERNEL_TRACE", "0") == "1"
    res = run_bass_kernel_spmd(nc, in_maps, core_ids=list(range(NCORES)), trace=trace)
    LAST_RESULTS = res
    outs = [np.asarray(r["out"]).reshape(B_LOC, 1) for r in res.results]
    return np.concatenate(outs, axis=0).astype(np.float32)
